# revision 36
# baseline (speedup 1.0000x reference)
"""Multi-head causal attention (B=4, T=2048, C=1024, H=16, DH=64) on 8 TRN2 cores.

Sharding: core = (batch b, head-half). Each core computes 8 heads of batch b
and a partial output projection (its 512 rows of Wo); the host sums the two
partials per batch and adds bo.

v2 (fp8): QKV projections and the AV matmul run in fp8e4 DoubleRow perf mode
(2 k-chunks per matmul, ~1.77x PE stream rate); scores stay bf16 with the two
heads' K=64 matmuls running concurrently on row-groups 0/64. Softmax exp is a
one-instruction Schraudolph approximation: u8 = trunc(A*score + B) IS the
fp8e4m3 bit pattern of exp(score*scale), emitted on ScalarE/DVE/GpSimd in
rotation straight from score PSUM. The denominator comes for free as row 64
of the AV matmul (V augmented with a ones column), so numerator and
denominator use identical quantized weights and the approximation bias
cancels. Rows t<128 have too few softmax terms to average the fp8 noise
away, so the first 128 columns of q/k/v are recomputed in bf16 and the
(t<128, s<128) block uses exact spline exp + a bf16 AV matmul.
"""

import numpy as np
import ml_dtypes

import concourse.bass as bass
import concourse.tile as tile
from concourse import bacc, mybir
import concourse.bass_utils as bass_utils

# Problem shapes (hardcoded; kernel.py must be self-contained).
H, DH, C = 16, 64, 1024
B, T = 4, 2048
N_CORES = 8
HPC = 8            # heads per core
NPAIR = HPC // 2   # head pairs per core
P = 128
CCH = C // P       # 8 contraction chunks of 128
TT = 512           # t tile width (attention + projections)
NT = T // TT       # 4
NSB = T // P       # 16 s blocks
NVP = NSB // 2     # 8 s-chunk pairs (DoubleRow)
VW = 72            # per-head v2 row pitch (65 used, padded for 16B stride)
SCALE = 1.0 / 8.0  # 1/sqrt(DH)
# Schraudolph exp-to-fp8e4m3: byte = trunc(score*A + B) viewed as fp8.
EXP_A = float((8.0 / np.log(2.0)) * SCALE)
EXP_B = 56.06
EXP_C = EXP_B / EXP_A  # fused-mask form: byte = (score + C) * mask(A or 0)
F32 = mybir.dt.float32
BF16 = mybir.dt.bfloat16
F8 = mybir.dt.float8e4

_CACHE = {}


def _build():
    """Emit the Bass/Tile program (identical for every core)."""
    from contextlib import ExitStack

    nc = bacc.Bacc("TRN2", target_bir_lowering=False, debug=False)
    xt_d = nc.dram_tensor("xt", [NT, P, CCH, TT], F8, kind="ExternalInput").ap()
    wq_d = nc.dram_tensor("wq", [NPAIR, P, CCH, P], F8, kind="ExternalInput").ap()
    wk_d = nc.dram_tensor("wk", [NPAIR, P, CCH, P], F8, kind="ExternalInput").ap()
    wv_d = nc.dram_tensor("wv", [P, CCH, HPC * DH], F8, kind="ExternalInput").ap()
    xb_d = nc.dram_tensor("xb", [P, CCH, P], BF16, kind="ExternalInput").ap()
    wqb_d = nc.dram_tensor("wqb", [NPAIR, P, CCH, P], BF16, kind="ExternalInput").ap()
    wkb_d = nc.dram_tensor("wkb", [NPAIR, P, CCH, P], BF16, kind="ExternalInput").ap()
    wvb_d = nc.dram_tensor("wvb", [P, CCH, HPC * DH], BF16, kind="ExternalInput").ap()
    bm_d = nc.dram_tensor("bm", [P, 2, TT], F32, kind="ExternalInput").ap()
    wo_d = nc.dram_tensor("wo", [HPC * DH, C], BF16, kind="ExternalInput").ap()
    y_d = nc.dram_tensor("y", [T, C], F32, kind="ExternalOutput").ap()
    # DRAM bounce rows for softmax-denominator partition-broadcast
    rb_d = nc.dram_tensor("rbounce", [NPAIR * NT * 2, TT], F32).ap()

    with tile.TileContext(nc) as tc, ExitStack() as ctx:
        # ---- persistent SBUF tensors ----
        persist = ctx.enter_context(tc.tile_pool(name="persist", bufs=1))
        ypool = ctx.enter_context(tc.tile_pool(name="yout", bufs=4))
        qT = [persist.tile([P, T], BF16, name=f"qT{p}", tag=f"qT{p}") for p in range(NPAIR)]
        kT = [persist.tile([P, T], BF16, name=f"kT{p}", tag=f"kT{p}") for p in range(NPAIR)]
        v2 = [persist.tile([P, 2, HPC, VW], F8, name=f"v2_{m}", tag=f"v2_{m}")
              for m in range(NVP)]
        vb0 = persist.tile([P, HPC, DH + 1], BF16, name="vb0", tag="vb0")
        oT = [persist.tile([P, T], BF16, name=f"oT{p}", tag=f"oT{p}")
              for p in range(NPAIR)]
        wo_s = [persist.tile([P, C], BF16, name=f"wo{c}", tag=f"wo{c}")
                for c in range(NPAIR)]

        with tc.tile_pool(name="wqkv", bufs=1) as wpool, \
             tc.tile_pool(name="xin", bufs=1) as xpool, \
             tc.tile_pool(name="st_ps", bufs=2, space="PSUM") as stp, \
             tc.tile_pool(name="po_ps", bufs=3, space="PSUM") as pop, \
             tc.tile_pool(name="hb_ps", bufs=1, space="PSUM") as hbp, \
             tc.tile_pool(name="est", bufs=4) as estp, \
             tc.tile_pool(name="estbf", bufs=2) as estbfp, \
             tc.tile_pool(name="sfx", bufs=4) as sfx:
            wq8 = wpool.tile([P, NPAIR, CCH, P], F8, name="wq8", tag="wq8")
            wk8 = wpool.tile([P, NPAIR, CCH, P], F8, name="wk8", tag="wk8")
            wv8 = wpool.tile([P, CCH, HPC * DH], F8, name="wv8", tag="wv8")
            wqb = wpool.tile([P, NPAIR, CCH, P], BF16, name="wqb", tag="wqb")
            wkb = wpool.tile([P, NPAIR, CCH, P], BF16, name="wkb", tag="wkb")
            wvb = wpool.tile([P, CCH, HPC * DH], BF16, name="wvb", tag="wvb")
            xt8 = xpool.tile([P, CCH, T], F8, tag="xt8")
            xb0 = xpool.tile([P, CCH, P], BF16, tag="xb0")
            bm = xpool.tile([P, 2, TT], F32, tag="bm")

            # DMA priority order: everything pair 0 / tile 0 needs first.
            nc.sync.dma_start(out=wq8[:, 0], in_=wq_d[0])
            nc.sync.dma_start(out=wk8[:, 0], in_=wk_d[0])
            nc.sync.dma_start(out=xt8[:, :, 0:TT], in_=xt_d[0])
            nc.sync.dma_start(out=bm, in_=bm_d)
            nc.sync.dma_start(out=xb0, in_=xb_d)
            nc.sync.dma_start(out=wqb[:, 0], in_=wqb_d[0])
            nc.sync.dma_start(out=wkb[:, 0], in_=wkb_d[0])
            nc.sync.dma_start(out=wv8, in_=wv_d)
            nc.sync.dma_start(out=wvb, in_=wvb_d)
            for j in range(1, NT):
                nc.sync.dma_start(
                    out=xt8[:, :, j * TT:(j + 1) * TT], in_=xt_d[j])
            for pr in range(1, NPAIR):
                nc.sync.dma_start(out=wq8[:, pr], in_=wq_d[pr])
                nc.sync.dma_start(out=wk8[:, pr], in_=wk_d[pr])
                nc.sync.dma_start(out=wqb[:, pr], in_=wqb_d[pr])
                nc.sync.dma_start(out=wkb[:, pr], in_=wkb_d[pr])
            for c in range(NPAIR):
                nc.sync.dma_start(out=wo_s[c], in_=wo_d[c * P:(c + 1) * P, :])

            # ones columns of the augmented V (row 64 of AV = softmax denom)
            for m in range(NVP):
                nc.vector.memset(v2[m][:, :, :, DH:DH + 1], 1.0)
            nc.vector.memset(vb0[:, :, DH:DH + 1], 1.0)

            # HAM warmup: keep the PE busy with throwaway matmuls while the
            # first DMAs land, so real matmuls start at 2.4GHz.
            junk = wpool.tile([P, 256], BF16, name="junk", tag="junk")
            nc.vector.memset(junk, 0.5)
            jp = hbp.tile([P, TT], F32, tag="hb", name="jp")
            for _w in range(40):
                nc.tensor.matmul(out=jp[0:16, 0:256], lhsT=junk[:, 0:16],
                                 rhs=junk, start=(_w == 0), stop=(_w == 39))
            nc.vector.tensor_copy(junk[0:1, 0:16], jp[0:1, 0:16])

            def emit_heartbeat():
                """Tiny dependency-free matmul. One per attention chunk keeps
                the PE's HAM activity monitor at K=8/8 (2.4GHz) across the
                short est-chain stalls; a single ~3.4us-idle window would
                re-throttle the PE to 1.2GHz and double every MM's cost."""
                nc.tensor.matmul(out=jp[0:16, 0:16], lhsT=junk[:, 0:16],
                                 rhs=junk[:, 0:16], start=True, stop=True)

            def emit_v(s_idx):
                """V projection for s-block s_idx -> v2 (fp8). s_idx 0 runs in
                bf16 (from xb0/wvb) and also fills vb0."""
                pst = stp.tile([P, 2, TT], F32, tag="st", name="psv")
                ps = pst[:, 0, :]
                if s_idx == 0:
                    for c in range(CCH):
                        nc.tensor.matmul(
                            out=ps, lhsT=xb0[:, c, :], rhs=wvb[:, c, :],
                            start=(c == 0), stop=(c == CCH - 1))
                else:
                    for c in range(CCH // 2):
                        nc.tensor.matmul(
                            out=ps,
                            lhsT=xt8[:, 2 * c:2 * c + 2,
                                     s_idx * P:(s_idx + 1) * P],
                            rhs=wv8[:, 2 * c:2 * c + 2, :],
                            start=(c == 0), stop=(c == CCH // 2 - 1),
                            perf_mode=mybir.MatmulPerfMode.DoubleRow)
                nc.scalar.copy(
                    v2[s_idx // 2][:, s_idx % 2, :, 0:DH],
                    ps.rearrange("p (h d) -> p h d", h=HPC))
                if s_idx == 0:
                    nc.scalar.copy(
                        vb0[:, :, 0:DH],
                        ps.rearrange("p (h d) -> p h d", h=HPC))

            def emit_proj(w8, dstT, p, j):
                pst = stp.tile([P, 2, TT], F32, tag="st", name="psqk")
                ps = pst[:, 0, :]
                for c in range(CCH // 2):
                    nc.tensor.matmul(
                        out=ps,
                        lhsT=w8[:, p, 2 * c:2 * c + 2, :],
                        rhs=xt8[:, 2 * c:2 * c + 2, j * TT:(j + 1) * TT],
                        start=(c == 0), stop=(c == CCH // 2 - 1),
                        perf_mode=mybir.MatmulPerfMode.DoubleRow)
                nc.scalar.copy(dstT[p][:, j * TT:(j + 1) * TT], ps)

            def emit_patch(wb, dstT, p):
                """bf16 recompute of columns [0,128) of q/k tile j=0."""
                pst = stp.tile([P, 2, TT], F32, tag="st", name="pspt")
                ps = pst[:, 0, 0:P]
                for c in range(CCH):
                    nc.tensor.matmul(
                        out=ps, lhsT=wb[:, p, c, :], rhs=xb0[:, c, :],
                        start=(c == 0), stop=(c == CCH - 1))
                nc.scalar.copy(dstT[p][:, 0:P], ps)

            def emit_q(p, j):
                emit_proj(wq8, qT, p, j)

            def emit_k(p, j):
                emit_proj(wk8, kT, p, j)

            def emit_wo(j, pool):
                for tb in range(4 * j, 4 * j + 4):
                    # both j2 halves accumulate together so each oT lhsT is
                    # loaded once and reused for two matmuls
                    pss = [pool.tile([P, TT], F32, tag="py", name=f"psy{_j}")
                           for _j in range(C // TT)]
                    for c in range(NPAIR):
                        for j2 in range(C // TT):
                            nc.tensor.matmul(
                                out=pss[j2],
                                lhsT=oT[c][:, tb * P:(tb + 1) * P],
                                rhs=wo_s[c][:, j2 * TT:(j2 + 1) * TT],
                                start=(c == 0), stop=(c == NPAIR - 1))
                    for j2 in range(C // TT):
                        yt = ypool.tile([P, TT], F32, tag="yt")
                        nc.scalar.copy(yt, pss[j2])
                        nc.sync.dma_start(
                            out=y_d[tb * P:(tb + 1) * P, j2 * TT:(j2 + 1) * TT],
                            in_=yt)

            exp_rot = [0]

            def emit_exp_full_hh(hh, dst_ap, src_ap):
                """Off-diagonal chunk, one head: hh0 on ScalarE (spline);
                hh1 alternates ScalarE/DVE to balance engine load while
                keeping per-chunk exp latency ~one sub-instruction."""
                if hh == 0:
                    use_scalar = True
                else:
                    use_scalar = (exp_rot[0] % 2 == 1)
                    exp_rot[0] += 1
                if use_scalar:
                    nc.scalar.activation(
                        dst_ap, src_ap,
                        mybir.ActivationFunctionType.Exp, scale=SCALE)
                else:
                    nc.vector.tensor_scalar(
                        out=dst_ap.bitcast(mybir.dt.uint8), in0=src_ap,
                        scalar1=EXP_A, scalar2=EXP_B,
                        op0=mybir.AluOpType.mult, op1=mybir.AluOpType.add)

            def emit_exp_diag(dst_ap, src_ap, fd):
                """Diagonal chunk: ONE DVE op fuses Schraudolph exp with the
                causal mask: byte = (score + C) * bm, where bm holds A on
                valid positions and 0 above the diagonal, so masked entries
                become exact fp8 zero. No gpsimd hop in the chain."""
                nc.vector.scalar_tensor_tensor(
                    out=dst_ap.bitcast(mybir.dt.uint8), in0=src_ap,
                    scalar=EXP_C, in1=bm[:, :, 0:fd],
                    op0=mybir.AluOpType.add, op1=mybir.AluOpType.mult)

            def emit_attn(p, j, prefills=None):
                nchunk = 4 * j + 4  # causal: s chunks 0 .. 4j+3
                npair = nchunk // 2
                po = [pop.tile([DH + 1, TT], F32, name=f"po{_hh}", tag="po")
                      for _hh in range(2)]
                ests = {}
                ebf_box = [None]

                def emit_av(m):
                    # AV for the chunk pair (2m, 2m+1) in fp8 DoubleRow.
                    # Diagonal pairs: DR over the both-valid column range
                    # plus a solo matmul for chunk 2m's exclusive range.
                    est = ests.pop(m)
                    f0A = max(0, P * (2 * m - 4 * j))
                    f0B = max(0, P * (2 * m + 1 - 4 * j))
                    for hh in range(2):
                        h = p * 2 + hh
                        if f0B > f0A:  # solo piece for chunk 2m
                            if j == 0 and m == 0:
                                nc.tensor.matmul(
                                    out=po[hh][:, f0A:f0B],
                                    lhsT=vb0[:, h, :],
                                    rhs=ebf_box[0][:, hh, :],
                                    start=(m == 0), stop=False)
                            else:
                                nc.tensor.matmul(
                                    out=po[hh][:, f0A:f0B],
                                    lhsT=v2[m][:, 0, h, 0:DH + 1],
                                    rhs=est[:, 0, hh, f0A:f0B],
                                    start=(m == 0), stop=False)
                        nc.tensor.matmul(
                            out=po[hh][:, f0B:TT],
                            lhsT=v2[m][:, :, h, 0:DH + 1],
                            rhs=est[:, :, hh, f0B:TT],
                            start=(m == 0 and f0B == f0A),
                            stop=(m == npair - 1),
                            perf_mode=mybir.MatmulPerfMode.DoubleRow)

                for c in range(nchunk):
                    # diagonal-crossing chunks (c >= 4j) only have valid
                    # scores at t-columns f >= 128*(c-4j)
                    f0 = max(0, P * (c - 4 * j))
                    par = c % 2
                    m = c // 2
                    if par == 0:
                        ests[m] = estp.tile([P, 2, 2, TT], F8, tag="est",
                                            name=f"est{m}")
                    est = ests[m]
                    st = stp.tile([P, 2, TT], F32, tag="st")
                    for hh in range(2):
                        r0 = hh * DH
                        nc.tensor.matmul(
                            out=st[:, hh, f0:TT],
                            lhsT=kT[p][r0:r0 + DH, c * P:(c + 1) * P],
                            rhs=qT[p][r0:r0 + DH, j * TT + f0:(j + 1) * TT],
                            start=True, stop=True)
                    first0 = (j == 0 and c == 0)
                    if first0:
                        # exact bf16 path for the (t<128, s<128) block; fp8
                        # spline only for t >= 128 (no mask needed there)
                        ebf = estbfp.tile([P, 2, P], BF16, tag="ebf")
                        ebf_box[0] = ebf
                        nc.scalar.activation(
                            ebf, st[:, :, 0:P],
                            mybir.ActivationFunctionType.Exp, scale=SCALE)
                        for hh in range(2):
                            emit_exp_full_hh(hh, est[:, 0, hh, P:TT],
                                             st[:, hh, P:TT])
                        for hh in range(2):
                            # keep where col_idx - p >= 0
                            nc.gpsimd.affine_select(
                                out=ebf[:, hh, :], in_=ebf[:, hh, :],
                                compare_op=mybir.AluOpType.is_ge,
                                fill=0.0, base=0,
                                pattern=[[1, P]], channel_multiplier=-1)
                    elif c >= 4 * j:
                        emit_exp_diag(est[:, par, :, f0:TT],
                                      st[:, :, f0:TT], TT - f0)
                    else:
                        for hh in range(2):
                            emit_exp_full_hh(hh, est[:, par, hh, 0:TT],
                                             st[:, hh, :])
                    emit_heartbeat()
                    if prefills and c in prefills:
                        for fn in prefills[c]:
                            fn()
                    # software pipeline: AV for pair m-1 issues after pair
                    # m's scores, so its exp has a full pair of PE work to
                    # hide behind and the PE never waits on ScalarE/DVE.
                    if par == 1 and m >= 1:
                        emit_av(m - 1)
                emit_av(npair - 1)
                # normalize: rows 0..63 = unnormalized o^T, row 64 = denom
                rs2 = sfx.tile([33, TT], F32, tag="rs2")
                for hh in range(2):
                    nc.scalar.copy(rs2[32 * hh:32 * hh + 1, :],
                                   po[hh][DH:DH + 1, :])
                rec = sfx.tile([33, TT], F32, tag="rec")
                rscr = sfx.tile([33, TT], F32, tag="rscr")
                # rows 1..31 are garbage; one batched reciprocal, only rows
                # 0 and 32 are consumed.
                nc.vector.reciprocal_approx_accurate(rec, rs2, rscr)
                for hh in range(2):
                    r = (p * NT + j) * 2 + hh
                    nc.sync.dma_start(out=rb_d[r:r + 1, :],
                                      in_=rec[32 * hh:32 * hh + 1, :])
                    bc = sfx.tile([DH, TT], F32, name=f"bc{hh}", tag=f"bc{hh}")
                    rb_row = rb_d[r:r + 1, :]
                    bcast = bass.AP(tensor=rb_row.tensor, offset=rb_row.offset,
                                    ap=[[0, DH]] + [list(a) for a in rb_row.ap[1:]])
                    nc.sync.dma_start(out=bc, in_=bcast)
                    # multiply straight out of PSUM; no staging copy
                    nc.vector.tensor_tensor(
                        out=oT[p][hh * DH:(hh + 1) * DH, j * TT:(j + 1) * TT],
                        in0=po[hh][0:DH, :], in1=bc,
                        op=mybir.AluOpType.mult)

            # Interleave: v-blocks / next-tile q,k / next-pair tile-0 q,k are
            # emitted as prefills inside the attention inner loop, placed on
            # the LAST chunks of each tile so the PE has filler work while
            # the final pair's exp drains on ScalarE/DVE.
            emit_q(0, 0)
            emit_k(0, 0)
            emit_patch(wqb, qT, 0)
            emit_patch(wkb, kT, 0)
            for p in range(NPAIR):
                for j in range(NT):
                    pre = {4 * j + i: [] for i in range(4)}
                    if p == 0:
                        for i in range(4):
                            pre[4 * j + i].append(lambda s=4 * j + i: emit_v(s))
                    if j + 1 < NT:
                        pre[4 * j + 1].append(lambda pp=p, jj=j + 1: emit_q(pp, jj))
                        pre[4 * j + 2].append(lambda pp=p, jj=j + 1: emit_k(pp, jj))
                    elif p + 1 < NPAIR:
                        pre[4 * j + 0].append(lambda pp=p + 1: emit_q(pp, 0))
                        pre[4 * j + 1].append(lambda pp=p + 1: emit_k(pp, 0))
                        pre[4 * j + 2].append(lambda pp=p + 1: emit_patch(wqb, qT, pp))
                        pre[4 * j + 3].append(lambda pp=p + 1: emit_patch(wkb, kT, pp))
                    emit_attn(p, j, prefills=pre)

        with tc.tile_pool(name="ps4", bufs=4, space="PSUM") as ps4:
            for j in range(NT):
                emit_wo(j, ps4)

    nc.compile()
    return nc


def _get_nc():
    if "nc" not in _CACHE:
        _CACHE["nc"] = _build()
    return _CACHE["nc"]


def _shard(x, Wq, Wk, Wv, Wo):
    """Per-core input dicts: core = 2*b + half."""
    f8 = ml_dtypes.float8_e4m3fn
    bf = ml_dtypes.bfloat16
    in_maps = []
    for core in range(N_CORES):
        b, half = divmod(core, 2)
        hs = slice(half * HPC, (half + 1) * HPC)
        # [H_c, C, DH] -> [C, H_c*DH] with column h*DH+d
        wq = np.ascontiguousarray(
            np.transpose(Wq[hs], (1, 0, 2)).reshape(C, HPC * DH))
        wk = np.ascontiguousarray(
            np.transpose(Wk[hs], (1, 0, 2)).reshape(C, HPC * DH))
        wv = np.ascontiguousarray(
            np.transpose(Wv[hs], (1, 0, 2)).reshape(C, HPC * DH))
        xt = np.ascontiguousarray(x[b].T)                      # [C, T]
        # xt fp8, j-major SBUF layout: [NT, P, CCH, TT]
        xt8 = np.ascontiguousarray(
            xt.reshape(CCH, P, NT, TT).transpose(2, 1, 0, 3)).astype(f8)
        # q/k weights fp8, pair-major: [NPAIR, P, CCH, P]
        def wlay(w, dt):
            return np.ascontiguousarray(
                w.reshape(CCH, P, NPAIR, P).transpose(2, 1, 0, 3)).astype(dt)
        # v weights: [P, CCH, HPC*DH]
        def vlay(w, dt):
            return np.ascontiguousarray(
                w.reshape(CCH, P, HPC * DH).transpose(1, 0, 2)).astype(dt)
        # bf16 x^T block for t<128 patches: [P, CCH, P]
        xb0 = np.ascontiguousarray(
            xt[:, 0:P].reshape(CCH, P, P).transpose(1, 0, 2)).astype(bf)
        # causal exp-mask constant: A on valid (col >= partition), 0 above
        bm1 = np.where(np.arange(TT)[None, :] >= np.arange(P)[:, None],
                       np.float32(EXP_A), np.float32(0.0))
        bm = np.ascontiguousarray(
            np.broadcast_to(bm1[:, None, :], (P, 2, TT))).astype(np.float32)
        in_maps.append({
            "xt": xt8,
            "wq": wlay(wq, f8), "wk": wlay(wk, f8), "wv": vlay(wv, f8),
            "xb": xb0, "bm": bm,
            "wqb": wlay(wq, bf), "wkb": wlay(wk, bf), "wvb": vlay(wv, bf),
            "wo": np.ascontiguousarray(
                Wo[half * HPC * DH:(half + 1) * HPC * DH, :]).astype(bf),
        })
    return in_maps


def _run(in_maps, trace=False):
    nc = _get_nc()
    return bass_utils.run_bass_kernel_spmd(
        nc, in_maps, core_ids=list(range(N_CORES)), trace=trace)


def _gather(results, bo):
    out = np.empty((B, T, C), dtype=np.float32)
    for b in range(B):
        out[b] = results[2 * b]["y"] + results[2 * b + 1]["y"] + bo
    return out


def kernel(x, Wq, Wk, Wv, Wo, bo):
    x = np.asarray(x, dtype=np.float32)
    res = _run(_shard(x, np.asarray(Wq), np.asarray(Wk),
                      np.asarray(Wv), np.asarray(Wo)))
    return _gather(res.results, np.asarray(bo, dtype=np.float32))


def kernel_traced(x, Wq, Wk, Wv, Wo, bo):
    """Like kernel() but captures an NTFF profile; returns (out, BassKernelResults)."""
    import sys, types
    if "antenv.axon_hooks" not in sys.modules:
        mod = types.ModuleType("antenv.axon_hooks")
        _state = {"hook": None}
        mod.set_axon_ntff_profile_hook = lambda h: _state.__setitem__("hook", h)
        mod.get_axon_ntff_profile_hook = lambda: _state["hook"]
        sys.modules["antenv.axon_hooks"] = mod
        from trn_agent_boot.trn_boot import _ntff_profile_via_ctypes
        mod.set_axon_ntff_profile_hook(
            _ntff_profile_via_ctypes("/opt/axon/libaxon_pjrt.so"))
    bass_utils.upload_artifacts = lambda tmpdir: "local://" + tmpdir
    x = np.asarray(x, dtype=np.float32)
    res = _run(_shard(x, np.asarray(Wq), np.asarray(Wk),
                      np.asarray(Wv), np.asarray(Wo)), trace=True)
    return _gather(res.results, np.asarray(bo, dtype=np.float32)), res


# revision 44
# speedup vs baseline: 1.2505x; 1.2505x over previous
"""Multi-head causal attention (B=4, T=2048, C=1024, H=16, DH=64) on 8 TRN2 cores.

Sharding: core = (batch b, head-half). Each core computes 8 heads of batch b
and a partial output projection (its 512 rows of Wo); the host sums the two
partials per batch and adds bo.

On-chip layout is fully "transposed": matmul computes out = lhsT.T @ rhs, so
we keep x^T, q^T, k^T resident with the contraction dim on partitions.
Scores are computed as ST[s, t] = k_s . q_t (contraction d=64, two heads
row-tiled onto the 128-row PE array). Softmax runs without max-subtraction
(scores are bounded ~ +-5 for this input distribution): exp on ScalarE reads
PSUM directly, causal zeroing via gpsimd.affine_select post-exp, and the
denominator comes for free as row 64 of the AV matmul (V is augmented with a
ones column, M=65).
"""

import numpy as np
import ml_dtypes

import concourse.bass as bass
import concourse.tile as tile
from concourse import bacc, mybir
import concourse.bass_utils as bass_utils

# Problem shapes (hardcoded; kernel.py must be self-contained).
H, DH, C = 16, 64, 1024
B, T = 4, 2048
N_CORES = 8
HPC = 8            # heads per core
NPAIR = HPC // 2   # head pairs per core
P = 128
CCH = C // P       # 8 contraction chunks of 128
TT = 512           # t tile width (attention + projections)
NT = T // TT       # 4
NSB = T // P       # 16 s blocks
SCALE = 1.0 / 8.0  # 1/sqrt(DH)
F32 = mybir.dt.float32
BF16 = mybir.dt.bfloat16
F8 = mybir.dt.float8e4

_CACHE = {}


def _build():
    """Emit the Bass/Tile program (identical for every core)."""
    from contextlib import ExitStack

    nc = bacc.Bacc("TRN2", target_bir_lowering=False, debug=False)
    # fp8 inputs in SBUF-friendly layouts (projections run fp8 DoubleRow);
    # bf16 copies of x^T cols [0,128) and the full weights patch the first
    # 128 rows of q/k/v, whose softmax rows average too few terms to wash
    # out fp8 projection noise.
    xt_d = nc.dram_tensor("xt", [NT, P, CCH, TT], F8, kind="ExternalInput").ap()
    wq_d = nc.dram_tensor("wq", [NPAIR, P, CCH, P], F8, kind="ExternalInput").ap()
    wk_d = nc.dram_tensor("wk", [NPAIR, P, CCH, P], F8, kind="ExternalInput").ap()
    wv_d = nc.dram_tensor("wv", [P, CCH, HPC * DH], F8, kind="ExternalInput").ap()
    xb_d = nc.dram_tensor("xb", [P, CCH, P], BF16, kind="ExternalInput").ap()
    wqb_d = nc.dram_tensor("wqb", [NPAIR, P, CCH, P], BF16, kind="ExternalInput").ap()
    wkb_d = nc.dram_tensor("wkb", [NPAIR, P, CCH, P], BF16, kind="ExternalInput").ap()
    wvb_d = nc.dram_tensor("wvb", [P, CCH, HPC * DH], BF16, kind="ExternalInput").ap()
    wo_d = nc.dram_tensor("wo", [HPC * DH, C], BF16, kind="ExternalInput").ap()
    y_d = nc.dram_tensor("y", [T, C], F32, kind="ExternalOutput").ap()
    # DRAM bounce rows for softmax-denominator partition-broadcast
    rb_d = nc.dram_tensor("rbounce", [NPAIR * NT * 2, TT], F32).ap()

    with tile.TileContext(nc) as tc, ExitStack() as ctx:
        # ---- persistent SBUF tensors ----
        persist = ctx.enter_context(tc.tile_pool(name="persist", bufs=1))
        ypool = ctx.enter_context(tc.tile_pool(name="yout", bufs=4))
        qT = [persist.tile([P, T], BF16, name=f"qT{p}", tag=f"qT{p}") for p in range(NPAIR)]
        kT = [persist.tile([P, T], BF16, name=f"kT{p}", tag=f"kT{p}") for p in range(NPAIR)]
        v_aug = [persist.tile([P, HPC, DH + 1], BF16, name=f"va{c}", tag=f"va{c}")
                 for c in range(NSB)]
        oT = [persist.tile([P, T], BF16, name=f"oT{p}", tag=f"oT{p}")
              for p in range(NPAIR)]
        wo_s = [persist.tile([P, C], BF16, name=f"wo{c}", tag=f"wo{c}")
                for c in range(NPAIR)]

        with tc.tile_pool(name="wqkv", bufs=1) as wpool, \
             tc.tile_pool(name="xin", bufs=1) as xpool, \
             tc.tile_pool(name="ps1", bufs=2, space="PSUM") as ps1, \
             tc.tile_pool(name="st_ps", bufs=2, space="PSUM") as stp, \
             tc.tile_pool(name="po_ps", bufs=2, space="PSUM") as pop, \
             tc.tile_pool(name="est", bufs=4) as estp, \
             tc.tile_pool(name="sfx", bufs=4) as sfx:
            wq8 = wpool.tile([P, NPAIR, CCH, P], F8, name="wq8", tag="wq8")
            wk8 = wpool.tile([P, NPAIR, CCH, P], F8, name="wk8", tag="wk8")
            wv8 = wpool.tile([P, CCH, HPC * DH], F8, name="wv8", tag="wv8")
            wqb = wpool.tile([P, NPAIR, CCH, P], BF16, name="wqb", tag="wqb")
            wkb = wpool.tile([P, NPAIR, CCH, P], BF16, name="wkb", tag="wkb")
            wvb = wpool.tile([P, CCH, HPC * DH], BF16, name="wvb", tag="wvb")
            xt8 = xpool.tile([P, CCH, T], F8, tag="xt8")
            xb0 = xpool.tile([P, CCH, P], BF16, tag="xb0")

            # DMA priority order: everything pair 0 / tile 0 needs first.
            # Host-side relayout makes every transfer contiguous per
            # partition (big packets, full DMA rate).
            nc.sync.dma_start(out=wq8[:, 0], in_=wq_d[0])
            nc.sync.dma_start(out=wk8[:, 0], in_=wk_d[0])
            nc.sync.dma_start(out=xt8[:, :, 0:TT], in_=xt_d[0])
            nc.sync.dma_start(out=xb0, in_=xb_d)
            nc.sync.dma_start(out=wqb[:, 0], in_=wqb_d[0])
            nc.sync.dma_start(out=wkb[:, 0], in_=wkb_d[0])
            nc.sync.dma_start(out=wv8, in_=wv_d)
            nc.sync.dma_start(out=wvb, in_=wvb_d)
            for j in range(1, NT):
                nc.sync.dma_start(
                    out=xt8[:, :, j * TT:(j + 1) * TT], in_=xt_d[j])
            for pr in range(1, NPAIR):
                nc.sync.dma_start(out=wq8[:, pr], in_=wq_d[pr])
                nc.sync.dma_start(out=wk8[:, pr], in_=wk_d[pr])
                nc.sync.dma_start(out=wqb[:, pr], in_=wqb_d[pr])
                nc.sync.dma_start(out=wkb[:, pr], in_=wkb_d[pr])
            for c in range(NPAIR):
                nc.sync.dma_start(out=wo_s[c], in_=wo_d[c * P:(c + 1) * P, :])

            # HAM warmup: keep the PE busy with throwaway matmuls while the
            # first DMAs land, so real matmuls start at 2.4GHz.
            junk = wpool.tile([P, 16], BF16, name="junk", tag="junk")
            nc.vector.memset(junk, 0.5)
            jps = ps1.tile([P, 16], F32, tag="p1", name="jps")
            for _w in range(60):
                nc.tensor.matmul(out=jps[0:16, :], lhsT=junk, rhs=junk,
                                 start=(_w == 0), stop=(_w == 59))
            nc.vector.tensor_copy(junk[0:1, :], jps[0:1, :])

            def emit_v(s_idx):
                """V projection for s-block s_idx (fp8 DoubleRow). Block 0
                runs in bf16 from xb0/wvb: its rows feed softmax rows with
                too few terms to average away fp8 noise."""
                ps = ps1.tile([P, TT], F32, tag="p1", name="psv")
                if s_idx == 0:
                    for c in range(CCH):
                        nc.tensor.matmul(
                            out=ps, lhsT=xb0[:, c, :], rhs=wvb[:, c, :],
                            start=(c == 0), stop=(c == CCH - 1))
                else:
                    for c in range(CCH // 2):
                        nc.tensor.matmul(
                            out=ps,
                            lhsT=xt8[:, 2 * c:2 * c + 2,
                                     s_idx * P:(s_idx + 1) * P],
                            rhs=wv8[:, 2 * c:2 * c + 2, :],
                            start=(c == 0), stop=(c == CCH // 2 - 1),
                            perf_mode=mybir.MatmulPerfMode.DoubleRow)
                nc.vector.tensor_copy(
                    v_aug[s_idx][:, :, 0:DH],
                    ps.rearrange("p (h d) -> p h d", h=HPC))
                nc.vector.memset(v_aug[s_idx][:, :, DH:DH + 1], 1.0)

            def emit_proj(w8, dstT, p, j):
                ps = ps1.tile([P, TT], F32, tag="p1", name="psqk")
                for c in range(CCH // 2):
                    nc.tensor.matmul(
                        out=ps,
                        lhsT=w8[:, p, 2 * c:2 * c + 2, :],
                        rhs=xt8[:, 2 * c:2 * c + 2, j * TT:(j + 1) * TT],
                        start=(c == 0), stop=(c == CCH // 2 - 1),
                        perf_mode=mybir.MatmulPerfMode.DoubleRow)
                nc.vector.tensor_copy(
                    dstT[p][:, j * TT:(j + 1) * TT], ps)

            def emit_patch(wb, dstT, p):
                """bf16 recompute of columns [0,128) of q/k tile j=0."""
                ps = ps1.tile([P, P], F32, tag="p1", name="pspt")
                for c in range(CCH):
                    nc.tensor.matmul(
                        out=ps, lhsT=wb[:, p, c, :], rhs=xb0[:, c, :],
                        start=(c == 0), stop=(c == CCH - 1))
                nc.vector.tensor_copy(dstT[p][:, 0:P], ps)

            def emit_q(p, j):
                emit_proj(wq8, qT, p, j)

            def emit_k(p, j):
                emit_proj(wk8, kT, p, j)

            def emit_wo(j, pool):
                for tb in range(4 * j, 4 * j + 4):
                    # both j2 halves accumulate together so each oT lhsT is
                    # loaded once and reused for two matmuls
                    pss = [pool.tile([P, TT], F32, tag="py", name=f"psy{_j}")
                           for _j in range(C // TT)]
                    for c in range(NPAIR):
                        for j2 in range(C // TT):
                            nc.tensor.matmul(
                                out=pss[j2],
                                lhsT=oT[c][:, tb * P:(tb + 1) * P],
                                rhs=wo_s[c][:, j2 * TT:(j2 + 1) * TT],
                                start=(c == 0), stop=(c == NPAIR - 1))
                    for j2 in range(C // TT):
                        yt = ypool.tile([P, TT], F32, tag="yt")
                        if (tb + j2) % 2 == 0:
                            nc.scalar.copy(yt, pss[j2])
                        else:
                            nc.vector.tensor_copy(yt, pss[j2])
                        nc.sync.dma_start(
                            out=y_d[tb * P:(tb + 1) * P, j2 * TT:(j2 + 1) * TT],
                            in_=yt)

            def emit_attn(p, j, fills=None, prefills=None):
                nchunk = 4 * j + 4  # causal: s chunks 0 .. 4j+3
                po = [pop.tile([DH + 1, TT], F32, name=f"po{_hh}", tag="po")
                      for _hh in range(2)]
                for c in range(nchunk):
                    # diagonal-crossing chunks (c >= 4j) only have valid
                    # scores at t-columns f >= 128*(c-4j); restrict QK, exp
                    # and AV to that range (the select zeroes the rest).
                    f0 = max(0, P * (c - 4 * j))
                    st = stp.tile([P, 2, TT], F32, tag="st")
                    for hh in range(2):
                        r0 = hh * DH
                        nc.tensor.matmul(
                            out=st[:, hh, f0:TT],
                            lhsT=kT[p][r0:r0 + DH, c * P:(c + 1) * P],
                            rhs=qT[p][r0:r0 + DH, j * TT + f0:(j + 1) * TT],
                            start=True, stop=True)
                    est = estp.tile([P, 2, TT], BF16, tag="est")
                    nc.scalar.activation(
                        est[:, :, f0:TT], st[:, :, f0:TT],
                        mybir.ActivationFunctionType.Exp,
                        scale=SCALE)
                    if prefills and c in prefills:
                        for fn in prefills[c]:
                            fn()
                    if c >= 4 * j:  # zero s > t inside the diagonal strip
                        # columns >= f0+128 are fully valid: f >= 128(k+1) >
                        # p + 128k for all p < 128, so only [f0, f0+128) needs
                        # the select.
                        k_off = c - 4 * j
                        for hh in range(2):
                            nc.gpsimd.affine_select(
                                out=est[:, hh, f0:f0 + P],
                                in_=est[:, hh, f0:f0 + P],
                                compare_op=mybir.AluOpType.is_ge,
                                fill=0.0, base=-(P * k_off) + f0,
                                pattern=[[1, P]], channel_multiplier=-1)
                    for hh in range(2):
                        h = p * 2 + hh
                        nc.tensor.matmul(
                            out=po[hh][:, f0:TT],
                            lhsT=v_aug[c][:, h, :],
                            rhs=est[:, hh, f0:TT],
                            start=(c == 0), stop=(c == nchunk - 1))
                    # low-priority PE fill emitted between attention chunks
                    if fills and c in fills:
                        for fn in fills[c]:
                            fn()
                # normalize: rows 0..63 = unnormalized o^T, row 64 = denom
                sAs = []
                rs2 = sfx.tile([33, TT], F32, tag="rs2")
                for hh in range(2):
                    sA = sfx.tile([DH, TT], F32, name=f"sA{hh}", tag=f"sA{hh}")
                    nc.vector.tensor_copy(sA, po[hh][0:DH, :])
                    nc.vector.tensor_copy(rs2[32 * hh:32 * hh + 1, :],
                                          po[hh][DH:DH + 1, :])
                    sAs.append(sA)
                rec = sfx.tile([33, TT], F32, tag="rec")
                rscr = sfx.tile([33, TT], F32, tag="rscr")
                # rows 1..31 are garbage; one batched reciprocal, only rows
                # 0 and 32 are consumed. approx_accurate is ~2 ULP, plenty
                # under the bf16 noise floor.
                nc.vector.reciprocal_approx_accurate(rec, rs2, rscr)
                for hh in range(2):
                    r = (p * NT + j) * 2 + hh
                    nc.sync.dma_start(out=rb_d[r:r + 1, :],
                                      in_=rec[32 * hh:32 * hh + 1, :])
                    bc = sfx.tile([DH, TT], F32, name=f"bc{hh}", tag=f"bc{hh}")
                    rb_row = rb_d[r:r + 1, :]
                    bcast = bass.AP(tensor=rb_row.tensor, offset=rb_row.offset,
                                    ap=[[0, DH]] + [list(a) for a in rb_row.ap[1:]])
                    nc.sync.dma_start(out=bc, in_=bcast)
                    nc.vector.tensor_mul(
                        oT[p][hh * DH:(hh + 1) * DH, j * TT:(j + 1) * TT],
                        sAs[hh], bc)

            # Coarse interleave: pair 0 carries the v-blocks between its
            # attention tiles; pairs 1-3 carry their own q/k tile.
            emit_q(0, 0)
            emit_k(0, 0)
            emit_patch(wqb, qT, 0)
            emit_patch(wkb, kT, 0)
            emit_attn(0, 0, prefills={
                i: [lambda s=i: emit_v(s)] for i in range(4)})
            for j in range(1, NT):
                emit_q(0, j)
                pre = {i: [lambda s=4 * j + i: emit_v(s)] for i in range(4)}
                pre[1] = pre[1] + [lambda jj=j: emit_k(0, jj)]
                emit_attn(0, j, prefills=pre)
            for p in range(1, NPAIR):
                for j in range(NT):
                    emit_q(p, j)
                    if j == 0:
                        emit_k(p, 0)
                        emit_patch(wqb, qT, p)
                        emit_patch(wkb, kT, p)
                        emit_attn(p, 0)
                    else:
                        emit_attn(p, j, prefills={
                            1: [lambda pp=p, jj=j: emit_k(pp, jj)]})

        with tc.tile_pool(name="ps4", bufs=4, space="PSUM") as ps4:
            for j in range(NT):
                emit_wo(j, ps4)

    nc.compile()
    return nc


def _get_nc():
    if "nc" not in _CACHE:
        _CACHE["nc"] = _build()
    return _CACHE["nc"]


def _shard(x, Wq, Wk, Wv, Wo):
    """Per-core input dicts: core = 2*b + half."""
    f8 = ml_dtypes.float8_e4m3fn
    bf = ml_dtypes.bfloat16
    in_maps = []
    for core in range(N_CORES):
        b, half = divmod(core, 2)
        hs = slice(half * HPC, (half + 1) * HPC)
        # [H_c, C, DH] -> [C, H_c*DH] with column h*DH+d
        wq = np.ascontiguousarray(
            np.transpose(Wq[hs], (1, 0, 2)).reshape(C, HPC * DH))
        wk = np.ascontiguousarray(
            np.transpose(Wk[hs], (1, 0, 2)).reshape(C, HPC * DH))
        wv = np.ascontiguousarray(
            np.transpose(Wv[hs], (1, 0, 2)).reshape(C, HPC * DH))
        xt = np.ascontiguousarray(x[b].T)                      # [C, T]
        # xt fp8, j-major SBUF layout: [NT, P, CCH, TT]
        xt8 = np.ascontiguousarray(
            xt.reshape(CCH, P, NT, TT).transpose(2, 1, 0, 3)).astype(f8)

        def wlay(w, dt):
            # q/k weights pair-major: [NPAIR, P, CCH, P]
            return np.ascontiguousarray(
                w.reshape(CCH, P, NPAIR, P).transpose(2, 1, 0, 3)).astype(dt)

        def vlay(w, dt):
            # v weights: [P, CCH, HPC*DH]
            return np.ascontiguousarray(
                w.reshape(CCH, P, HPC * DH).transpose(1, 0, 2)).astype(dt)

        # bf16 x^T block for the t<128 patches: [P, CCH, P]
        xb0 = np.ascontiguousarray(
            xt[:, 0:P].reshape(CCH, P, P).transpose(1, 0, 2)).astype(bf)
        in_maps.append({
            "xt": xt8,
            "wq": wlay(wq, f8), "wk": wlay(wk, f8), "wv": vlay(wv, f8),
            "xb": xb0,
            "wqb": wlay(wq, bf), "wkb": wlay(wk, bf), "wvb": vlay(wv, bf),
            "wo": np.ascontiguousarray(
                Wo[half * HPC * DH:(half + 1) * HPC * DH, :]).astype(bf),
        })
    return in_maps


def _run(in_maps, trace=False):
    nc = _get_nc()
    return bass_utils.run_bass_kernel_spmd(
        nc, in_maps, core_ids=list(range(N_CORES)), trace=trace)


def _gather(results, bo):
    out = np.empty((B, T, C), dtype=np.float32)
    for b in range(B):
        out[b] = results[2 * b]["y"] + results[2 * b + 1]["y"] + bo
    return out


def kernel(x, Wq, Wk, Wv, Wo, bo):
    x = np.asarray(x, dtype=np.float32)
    res = _run(_shard(x, np.asarray(Wq), np.asarray(Wk),
                      np.asarray(Wv), np.asarray(Wo)))
    return _gather(res.results, np.asarray(bo, dtype=np.float32))


def kernel_traced(x, Wq, Wk, Wv, Wo, bo):
    """Like kernel() but captures an NTFF profile; returns (out, BassKernelResults)."""
    import sys, types
    if "antenv.axon_hooks" not in sys.modules:
        mod = types.ModuleType("antenv.axon_hooks")
        _state = {"hook": None}
        mod.set_axon_ntff_profile_hook = lambda h: _state.__setitem__("hook", h)
        mod.get_axon_ntff_profile_hook = lambda: _state["hook"]
        sys.modules["antenv.axon_hooks"] = mod
        from trn_agent_boot.trn_boot import _ntff_profile_via_ctypes
        mod.set_axon_ntff_profile_hook(
            _ntff_profile_via_ctypes("/opt/axon/libaxon_pjrt.so"))
    bass_utils.upload_artifacts = lambda tmpdir: "local://" + tmpdir
    x = np.asarray(x, dtype=np.float32)
    res = _run(_shard(x, np.asarray(Wq), np.asarray(Wk),
                      np.asarray(Wv), np.asarray(Wo)), trace=True)
    return _gather(res.results, np.asarray(bo, dtype=np.float32)), res



# revision 47
# speedup vs baseline: 1.2879x; 1.0298x over previous
"""Multi-head causal attention (B=4, T=2048, C=1024, H=16, DH=64) on 8 TRN2 cores.

Sharding: core = (batch b, head-half). Each core computes 8 heads of batch b
and a partial output projection (its 512 rows of Wo); the host sums the two
partials per batch and adds bo.

On-chip layout is fully "transposed": matmul computes out = lhsT.T @ rhs, so
we keep x^T, q^T, k^T resident with the contraction dim on partitions.
Scores are computed as ST[s, t] = k_s . q_t (contraction d=64, two heads
row-tiled onto the 128-row PE array). Softmax runs without max-subtraction
(scores are bounded ~ +-5 for this input distribution): exp on ScalarE reads
PSUM directly, causal zeroing via gpsimd.affine_select post-exp, and the
denominator comes for free as row 64 of the AV matmul (V is augmented with a
ones column, M=65).

The QKV projections (half the PE stream) run in fp8e4m3 DoubleRow perf mode:
x^T and the projection weights are pre-quantized on the host into SBUF-ready
contiguous layouts, and each matmul contracts two 128-row k-chunks at once
(~1.9x the bf16 stream rate, measured ~203ns per 512-col matmul). Softmax
rows t<128 average too few terms to wash out fp8 projection noise, so bf16
copies of x^T columns [0,128) and of the weights recompute the first 128
columns of q/k and v s-block 0 exactly (rel err 3.3e-3 vs 2.7e-3 all-bf16).
Attention, softmax, and the output projection are unchanged bf16 - their
heavier PE stream keeps the HAM clock gate at 2.4GHz and self-hides the
ScalarE/DVE softmax latencies (an all-fp8 variant measured slower: too
little PE work to stay warm; see kernel_fp8_full.py.ref).
"""

import numpy as np
import ml_dtypes

import concourse.bass as bass
import concourse.tile as tile
from concourse import bacc, mybir
import concourse.bass_utils as bass_utils

# Problem shapes (hardcoded; kernel.py must be self-contained).
H, DH, C = 16, 64, 1024
B, T = 4, 2048
N_CORES = 8
HPC = 8            # heads per core
NPAIR = HPC // 2   # head pairs per core
P = 128
CCH = C // P       # 8 contraction chunks of 128
TT = 512           # t tile width (attention + projections)
NT = T // TT       # 4
NSB = T // P       # 16 s blocks
SCALE = 1.0 / 8.0  # 1/sqrt(DH)
F32 = mybir.dt.float32
BF16 = mybir.dt.bfloat16
F8 = mybir.dt.float8e4

_CACHE = {}


def _build():
    """Emit the Bass/Tile program (identical for every core)."""
    from contextlib import ExitStack

    nc = bacc.Bacc("TRN2", target_bir_lowering=False, debug=False)
    # fp8 inputs in SBUF-friendly layouts (projections run fp8 DoubleRow);
    # bf16 copies of x^T cols [0,128) and the full weights patch the first
    # 128 rows of q/k/v, whose softmax rows average too few terms to wash
    # out fp8 projection noise.
    xt_d = nc.dram_tensor("xt", [NT, P, CCH, TT], F8, kind="ExternalInput").ap()
    wq_d = nc.dram_tensor("wq", [NPAIR, P, CCH, P], F8, kind="ExternalInput").ap()
    wk_d = nc.dram_tensor("wk", [NPAIR, P, CCH, P], F8, kind="ExternalInput").ap()
    wv_d = nc.dram_tensor("wv", [P, CCH, HPC * DH], F8, kind="ExternalInput").ap()
    xb_d = nc.dram_tensor("xb", [P, CCH, P], BF16, kind="ExternalInput").ap()
    wqb_d = nc.dram_tensor("wqb", [NPAIR, P, CCH, P], BF16, kind="ExternalInput").ap()
    wkb_d = nc.dram_tensor("wkb", [NPAIR, P, CCH, P], BF16, kind="ExternalInput").ap()
    wvb_d = nc.dram_tensor("wvb", [P, CCH, HPC * DH], BF16, kind="ExternalInput").ap()
    wo_d = nc.dram_tensor("wo", [HPC * DH, C], BF16, kind="ExternalInput").ap()
    y_d = nc.dram_tensor("y", [T, C], F32, kind="ExternalOutput").ap()
    # DRAM bounce rows for softmax-denominator partition-broadcast
    rb_d = nc.dram_tensor("rbounce", [NPAIR * NT * 2, TT], F32).ap()

    with tile.TileContext(nc) as tc, ExitStack() as ctx:
        # ---- persistent SBUF tensors ----
        persist = ctx.enter_context(tc.tile_pool(name="persist", bufs=1))
        ypool = ctx.enter_context(tc.tile_pool(name="yout", bufs=4))
        qT = [persist.tile([P, T], BF16, name=f"qT{p}", tag=f"qT{p}") for p in range(NPAIR)]
        kT = [persist.tile([P, T], BF16, name=f"kT{p}", tag=f"kT{p}") for p in range(NPAIR)]
        v_aug = [persist.tile([P, HPC, DH + 1], BF16, name=f"va{c}", tag=f"va{c}")
                 for c in range(NSB)]
        oT = [persist.tile([P, T], BF16, name=f"oT{p}", tag=f"oT{p}")
              for p in range(NPAIR)]
        wo_s = [persist.tile([P, C], BF16, name=f"wo{c}", tag=f"wo{c}")
                for c in range(NPAIR)]

        with tc.tile_pool(name="wqkv", bufs=1) as wpool, \
             tc.tile_pool(name="xin", bufs=1) as xpool, \
             tc.tile_pool(name="ps1", bufs=2, space="PSUM") as ps1, \
             tc.tile_pool(name="st_ps", bufs=2, space="PSUM") as stp, \
             tc.tile_pool(name="po_ps", bufs=2, space="PSUM") as pop, \
             tc.tile_pool(name="est", bufs=4) as estp, \
             tc.tile_pool(name="sfx", bufs=4) as sfx:
            wq8 = wpool.tile([P, NPAIR, CCH, P], F8, name="wq8", tag="wq8")
            wk8 = wpool.tile([P, NPAIR, CCH, P], F8, name="wk8", tag="wk8")
            wv8 = wpool.tile([P, CCH, HPC * DH], F8, name="wv8", tag="wv8")
            wqb = wpool.tile([P, NPAIR, CCH, P], BF16, name="wqb", tag="wqb")
            wkb = wpool.tile([P, NPAIR, CCH, P], BF16, name="wkb", tag="wkb")
            wvb = wpool.tile([P, CCH, HPC * DH], BF16, name="wvb", tag="wvb")
            xt8 = xpool.tile([P, CCH, T], F8, tag="xt8")
            xb0 = xpool.tile([P, CCH, P], BF16, tag="xb0")

            # DMA priority order: everything pair 0 / tile 0 needs first.
            # Host-side relayout makes every transfer contiguous per
            # partition (big packets, full DMA rate).
            nc.sync.dma_start(out=wq8[:, 0], in_=wq_d[0])
            nc.sync.dma_start(out=wk8[:, 0], in_=wk_d[0])
            nc.sync.dma_start(out=xt8[:, :, 0:TT], in_=xt_d[0])
            nc.sync.dma_start(out=xb0, in_=xb_d)
            nc.sync.dma_start(out=wqb[:, 0], in_=wqb_d[0])
            nc.sync.dma_start(out=wkb[:, 0], in_=wkb_d[0])
            nc.sync.dma_start(out=wv8, in_=wv_d)
            nc.sync.dma_start(out=wvb, in_=wvb_d)
            for j in range(1, NT):
                nc.sync.dma_start(
                    out=xt8[:, :, j * TT:(j + 1) * TT], in_=xt_d[j])
            for pr in range(1, NPAIR):
                nc.sync.dma_start(out=wq8[:, pr], in_=wq_d[pr])
                nc.sync.dma_start(out=wk8[:, pr], in_=wk_d[pr])
                nc.sync.dma_start(out=wqb[:, pr], in_=wqb_d[pr])
                nc.sync.dma_start(out=wkb[:, pr], in_=wkb_d[pr])
            for c in range(NPAIR):
                nc.sync.dma_start(out=wo_s[c], in_=wo_d[c * P:(c + 1) * P, :])

            # HAM warmup: keep the PE busy with throwaway matmuls while the
            # first DMAs land (~8us), so real matmuls start at 2.4GHz and
            # the PE never sees a >3.4us idle window at kernel start.
            junk = wpool.tile([P, 256], BF16, name="junk", tag="junk")
            nc.vector.memset(junk, 0.5)
            jps = ps1.tile([P, 256], F32, tag="p1", name="jps")
            for _w in range(40):
                nc.tensor.matmul(out=jps[0:16, :], lhsT=junk[:, 0:16],
                                 rhs=junk, start=(_w == 0), stop=(_w == 39))
            nc.vector.tensor_copy(junk[0:1, 0:16], jps[0:1, 0:16])

            def emit_v(s_idx):
                """V projection for s-block s_idx (fp8 DoubleRow). Block 0
                runs in bf16 from xb0/wvb: its rows feed softmax rows with
                too few terms to average away fp8 noise."""
                ps = ps1.tile([P, TT], F32, tag="p1", name="psv")
                if s_idx == 0:
                    for c in range(CCH):
                        nc.tensor.matmul(
                            out=ps, lhsT=xb0[:, c, :], rhs=wvb[:, c, :],
                            start=(c == 0), stop=(c == CCH - 1))
                else:
                    for c in range(CCH // 2):
                        nc.tensor.matmul(
                            out=ps,
                            lhsT=xt8[:, 2 * c:2 * c + 2,
                                     s_idx * P:(s_idx + 1) * P],
                            rhs=wv8[:, 2 * c:2 * c + 2, :],
                            start=(c == 0), stop=(c == CCH // 2 - 1),
                            perf_mode=mybir.MatmulPerfMode.DoubleRow)
                nc.vector.tensor_copy(
                    v_aug[s_idx][:, :, 0:DH],
                    ps.rearrange("p (h d) -> p h d", h=HPC))
                nc.vector.memset(v_aug[s_idx][:, :, DH:DH + 1], 1.0)

            def emit_proj(w8, dstT, p, j):
                ps = ps1.tile([P, TT], F32, tag="p1", name="psqk")
                for c in range(CCH // 2):
                    nc.tensor.matmul(
                        out=ps,
                        lhsT=w8[:, p, 2 * c:2 * c + 2, :],
                        rhs=xt8[:, 2 * c:2 * c + 2, j * TT:(j + 1) * TT],
                        start=(c == 0), stop=(c == CCH // 2 - 1),
                        perf_mode=mybir.MatmulPerfMode.DoubleRow)
                nc.vector.tensor_copy(
                    dstT[p][:, j * TT:(j + 1) * TT], ps)

            def emit_patch(wb, dstT, p):
                """bf16 recompute of columns [0,128) of q/k tile j=0."""
                ps = ps1.tile([P, P], F32, tag="p1", name="pspt")
                for c in range(CCH):
                    nc.tensor.matmul(
                        out=ps, lhsT=wb[:, p, c, :], rhs=xb0[:, c, :],
                        start=(c == 0), stop=(c == CCH - 1))
                nc.vector.tensor_copy(dstT[p][:, 0:P], ps)

            def emit_q(p, j):
                emit_proj(wq8, qT, p, j)

            def emit_k(p, j):
                emit_proj(wk8, kT, p, j)

            def emit_wo(j, pool):
                for tb in range(4 * j, 4 * j + 4):
                    # both j2 halves accumulate together so each oT lhsT is
                    # loaded once and reused for two matmuls
                    pss = [pool.tile([P, TT], F32, tag="py", name=f"psy{_j}")
                           for _j in range(C // TT)]
                    for c in range(NPAIR):
                        for j2 in range(C // TT):
                            nc.tensor.matmul(
                                out=pss[j2],
                                lhsT=oT[c][:, tb * P:(tb + 1) * P],
                                rhs=wo_s[c][:, j2 * TT:(j2 + 1) * TT],
                                start=(c == 0), stop=(c == NPAIR - 1))
                    for j2 in range(C // TT):
                        yt = ypool.tile([P, TT], F32, tag="yt")
                        if (tb + j2) % 2 == 0:
                            nc.scalar.copy(yt, pss[j2])
                        else:
                            nc.vector.tensor_copy(yt, pss[j2])
                        nc.sync.dma_start(
                            out=y_d[tb * P:(tb + 1) * P, j2 * TT:(j2 + 1) * TT],
                            in_=yt)

            def emit_attn(p, j, fills=None, prefills=None):
                nchunk = 4 * j + 4  # causal: s chunks 0 .. 4j+3
                po = [pop.tile([DH + 1, TT], F32, name=f"po{_hh}", tag="po")
                      for _hh in range(2)]
                for c in range(nchunk):
                    # diagonal-crossing chunks (c >= 4j) only have valid
                    # scores at t-columns f >= 128*(c-4j); restrict QK, exp
                    # and AV to that range (the select zeroes the rest).
                    f0 = max(0, P * (c - 4 * j))
                    st = stp.tile([P, 2, TT], F32, tag="st")
                    for hh in range(2):
                        r0 = hh * DH
                        nc.tensor.matmul(
                            out=st[:, hh, f0:TT],
                            lhsT=kT[p][r0:r0 + DH, c * P:(c + 1) * P],
                            rhs=qT[p][r0:r0 + DH, j * TT + f0:(j + 1) * TT],
                            start=True, stop=True)
                    est = estp.tile([P, 2, TT], BF16, tag="est")
                    nc.scalar.activation(
                        est[:, :, f0:TT], st[:, :, f0:TT],
                        mybir.ActivationFunctionType.Exp,
                        scale=SCALE)
                    if prefills and c in prefills:
                        for fn in prefills[c]:
                            fn()
                    if c >= 4 * j:  # zero s > t inside the diagonal strip
                        # columns >= f0+128 are fully valid: f >= 128(k+1) >
                        # p + 128k for all p < 128, so only [f0, f0+128) needs
                        # the select.
                        k_off = c - 4 * j
                        for hh in range(2):
                            nc.gpsimd.affine_select(
                                out=est[:, hh, f0:f0 + P],
                                in_=est[:, hh, f0:f0 + P],
                                compare_op=mybir.AluOpType.is_ge,
                                fill=0.0, base=-(P * k_off) + f0,
                                pattern=[[1, P]], channel_multiplier=-1)
                    for hh in range(2):
                        h = p * 2 + hh
                        nc.tensor.matmul(
                            out=po[hh][:, f0:TT],
                            lhsT=v_aug[c][:, h, :],
                            rhs=est[:, hh, f0:TT],
                            start=(c == 0), stop=(c == nchunk - 1))
                    # low-priority PE fill emitted between attention chunks
                    if fills and c in fills:
                        for fn in fills[c]:
                            fn()
                # normalize: rows 0..63 = unnormalized o^T, row 64 = denom
                sAs = []
                rs2 = sfx.tile([33, TT], F32, tag="rs2")
                for hh in range(2):
                    sA = sfx.tile([DH, TT], F32, name=f"sA{hh}", tag=f"sA{hh}")
                    nc.vector.tensor_copy(sA, po[hh][0:DH, :])
                    nc.vector.tensor_copy(rs2[32 * hh:32 * hh + 1, :],
                                          po[hh][DH:DH + 1, :])
                    sAs.append(sA)
                rec = sfx.tile([33, TT], F32, tag="rec")
                rscr = sfx.tile([33, TT], F32, tag="rscr")
                # rows 1..31 are garbage; one batched reciprocal, only rows
                # 0 and 32 are consumed. approx_accurate is ~2 ULP, plenty
                # under the bf16 noise floor.
                nc.vector.reciprocal_approx_accurate(rec, rs2, rscr)
                for hh in range(2):
                    r = (p * NT + j) * 2 + hh
                    nc.sync.dma_start(out=rb_d[r:r + 1, :],
                                      in_=rec[32 * hh:32 * hh + 1, :])
                    bc = sfx.tile([DH, TT], F32, name=f"bc{hh}", tag=f"bc{hh}")
                    rb_row = rb_d[r:r + 1, :]
                    bcast = bass.AP(tensor=rb_row.tensor, offset=rb_row.offset,
                                    ap=[[0, DH]] + [list(a) for a in rb_row.ap[1:]])
                    nc.sync.dma_start(out=bc, in_=bcast)
                    nc.vector.tensor_mul(
                        oT[p][hh * DH:(hh + 1) * DH, j * TT:(j + 1) * TT],
                        sAs[hh], bc)

            # Coarse interleave: pair 0 carries the v-blocks between its
            # attention tiles; pairs 1-3 carry their own q/k tile.
            # Interleave: pair 0 carries the v-blocks; every pair prefills
            # its own next-tile q (and JIT k for the diagonal) inside the
            # current tile's attention, and the next pair's tile-0
            # projections+patches run under the current pair's last tile,
            # so no tile or pair transition exposes a projection copy.
            emit_q(0, 0)
            emit_k(0, 0)
            emit_patch(wqb, qT, 0)
            emit_patch(wkb, kT, 0)
            for p in range(NPAIR):
                for j in range(NT):
                    pre = {i: [] for i in range(4)}
                    if p == 0:
                        for i in range(4):
                            pre[i].append(lambda s=4 * j + i: emit_v(s))
                    if j >= 1:
                        pre[1].append(lambda pp=p, jj=j: emit_k(pp, jj))
                    if j + 1 < NT:
                        pre[2].append(lambda pp=p, jj=j + 1: emit_q(pp, jj))
                    elif p + 1 < NPAIR:
                        pre[12] = [lambda pp=p + 1: emit_q(pp, 0)]
                        pre[13] = [lambda pp=p + 1: emit_k(pp, 0)]
                        pre[14] = [lambda pp=p + 1: emit_patch(wqb, qT, pp)]
                        pre[15] = [lambda pp=p + 1: emit_patch(wkb, kT, pp)]
                    emit_attn(p, j, prefills=pre)

        with tc.tile_pool(name="ps4", bufs=4, space="PSUM") as ps4:
            for j in range(NT):
                emit_wo(j, ps4)

    nc.compile()
    return nc


def _get_nc():
    if "nc" not in _CACHE:
        _CACHE["nc"] = _build()
    return _CACHE["nc"]


def _shard(x, Wq, Wk, Wv, Wo):
    """Per-core input dicts: core = 2*b + half."""
    f8 = ml_dtypes.float8_e4m3fn
    bf = ml_dtypes.bfloat16
    in_maps = []
    for core in range(N_CORES):
        b, half = divmod(core, 2)
        hs = slice(half * HPC, (half + 1) * HPC)
        # [H_c, C, DH] -> [C, H_c*DH] with column h*DH+d
        wq = np.ascontiguousarray(
            np.transpose(Wq[hs], (1, 0, 2)).reshape(C, HPC * DH))
        wk = np.ascontiguousarray(
            np.transpose(Wk[hs], (1, 0, 2)).reshape(C, HPC * DH))
        wv = np.ascontiguousarray(
            np.transpose(Wv[hs], (1, 0, 2)).reshape(C, HPC * DH))
        xt = np.ascontiguousarray(x[b].T)                      # [C, T]
        # xt fp8, j-major SBUF layout: [NT, P, CCH, TT]
        xt8 = np.ascontiguousarray(
            xt.reshape(CCH, P, NT, TT).transpose(2, 1, 0, 3)).astype(f8)

        def wlay(w, dt):
            # q/k weights pair-major: [NPAIR, P, CCH, P]
            return np.ascontiguousarray(
                w.reshape(CCH, P, NPAIR, P).transpose(2, 1, 0, 3)).astype(dt)

        def vlay(w, dt):
            # v weights: [P, CCH, HPC*DH]
            return np.ascontiguousarray(
                w.reshape(CCH, P, HPC * DH).transpose(1, 0, 2)).astype(dt)

        # bf16 x^T block for the t<128 patches: [P, CCH, P]
        xb0 = np.ascontiguousarray(
            xt[:, 0:P].reshape(CCH, P, P).transpose(1, 0, 2)).astype(bf)
        in_maps.append({
            "xt": xt8,
            "wq": wlay(wq, f8), "wk": wlay(wk, f8), "wv": vlay(wv, f8),
            "xb": xb0,
            "wqb": wlay(wq, bf), "wkb": wlay(wk, bf), "wvb": vlay(wv, bf),
            "wo": np.ascontiguousarray(
                Wo[half * HPC * DH:(half + 1) * HPC * DH, :]).astype(bf),
        })
    return in_maps


def _run(in_maps, trace=False):
    nc = _get_nc()
    return bass_utils.run_bass_kernel_spmd(
        nc, in_maps, core_ids=list(range(N_CORES)), trace=trace)


def _gather(results, bo):
    out = np.empty((B, T, C), dtype=np.float32)
    for b in range(B):
        out[b] = results[2 * b]["y"] + results[2 * b + 1]["y"] + bo
    return out


def kernel(x, Wq, Wk, Wv, Wo, bo):
    x = np.asarray(x, dtype=np.float32)
    res = _run(_shard(x, np.asarray(Wq), np.asarray(Wk),
                      np.asarray(Wv), np.asarray(Wo)))
    return _gather(res.results, np.asarray(bo, dtype=np.float32))


def kernel_traced(x, Wq, Wk, Wv, Wo, bo):
    """Like kernel() but captures an NTFF profile; returns (out, BassKernelResults)."""
    import sys, types
    if "antenv.axon_hooks" not in sys.modules:
        mod = types.ModuleType("antenv.axon_hooks")
        _state = {"hook": None}
        mod.set_axon_ntff_profile_hook = lambda h: _state.__setitem__("hook", h)
        mod.get_axon_ntff_profile_hook = lambda: _state["hook"]
        sys.modules["antenv.axon_hooks"] = mod
        from trn_agent_boot.trn_boot import _ntff_profile_via_ctypes
        mod.set_axon_ntff_profile_hook(
            _ntff_profile_via_ctypes("/opt/axon/libaxon_pjrt.so"))
    bass_utils.upload_artifacts = lambda tmpdir: "local://" + tmpdir
    x = np.asarray(x, dtype=np.float32)
    res = _run(_shard(x, np.asarray(Wq), np.asarray(Wk),
                      np.asarray(Wv), np.asarray(Wo)), trace=True)
    return _gather(res.results, np.asarray(bo, dtype=np.float32)), res

